# revision 18
# baseline (speedup 1.0000x reference)
"""Trainium2 Bass kernel for nn_Projection: out = [(1-s)*x, s],
s = -(1-||x||^2)/(1+||x||^2) per row.

Identity used: with sq = sum(x^2), s = (sq-1)/(sq+1) = 1 - 2/(1+sq).
Let t = 2/(1+sq). Then out = [t*x, 1-t].

Sharding: pure data parallel over rows across 8 NeuronCores.
"""

import sys

for _p in ("/opt/trn_rl_repo", "/opt/trn_rl_repo/concourse"):
    if _p not in sys.path:
        sys.path.insert(0, _p)

import numpy as np

import concourse.bacc as bacc
import concourse.bass as bass
import concourse.tile as tile
from concourse import mybir
from concourse.bass_utils import run_bass_kernel_spmd
from concourse.tile import add_dep_helper

N, D = 1048576, 128
N_CORES = 8
R = N // N_CORES  # 131072 rows per core
P = 128           # partitions (rows per block)
E = D + 1         # output row width (129)


def build_nc(rows: int = R, blk: int = 16, mul_engine: str = "gpsimd",
             io_bufs: int = 7, tmp_bufs: int = 4):
    """Build the per-core Bass program: x[rows, D] -> out[rows, E].

    rows must be divisible by P*blk. Processes `blk` 128-row blocks per
    super-tile iteration (one ~1MiB DMA each way for blk=16).
    mul_engine: 'gpsimd' or 'vector' — engine for the big out = x*t multiply.
    """
    nblocks = rows // P
    assert nblocks % blk == 0
    niter = nblocks // blk

    nc = bacc.Bacc(trn_type="TRN2")
    x = nc.dram_tensor("x", [rows, D], mybir.dt.float32, kind="ExternalInput")
    out = nc.dram_tensor("out", [rows, E], mybir.dt.float32, kind="ExternalOutput")

    # row index = i*(P*blk) + p*blk + j -> partition p holds `blk`
    # CONSECUTIVE rows per iteration, so each partition's DMA chunk is one
    # contiguous 8KB span (1 descriptor/partition instead of blk small ones).
    xv = x.ap().rearrange("(c p j) d -> c p j d", p=P, j=blk)
    ov = out.ap().rearrange("(c p j) e -> c p j e", p=P, j=blk)

    PRE = min(4, niter)  # load prefetch distance (traced ahead of stores so
    #                      a store's sem wait never blocks upcoming loads in
    #                      the Sync engine's in-order issue queue)

    with tile.TileContext(nc) as tc:
        with (
            tc.tile_pool(name="io", bufs=io_bufs) as io_pool,
            tc.tile_pool(name="tmp", bufs=tmp_bufs) as tmp_pool,
            tc.tile_pool(name="small", bufs=8) as small_pool,
            tc.tile_pool(name="singles", bufs=1) as singles,
        ):
            half = singles.tile([P, 1], mybir.dt.float32)
            nc.vector.memset(half, 0.5)
            half_b = half[:, 0:1].broadcast_to([P, blk])

            pending = []

            def issue_load(i):
                # Loads issue from the ACT HWDGE ring (qActDynamicHW) so they
                # drain concurrently with stores on SP's ring (qSPDynamicHW) —
                # one ring serializes its DMAs. Safe on ACT: the only wait is
                # the x-slot release from io_bufs iterations ago.
                x_t = io_pool.tile([P, blk, D], mybir.dt.float32, tag="x")
                nc.scalar.dma_start(out=x_t, in_=xv[i])
                pending.append(x_t)

            for i in range(PRE):
                issue_load(i)

            for i in range(niter):
                if i + PRE < niter:
                    issue_load(i + PRE)
                x_t = pending.pop(0)

                # xsq = (x/sqrt(2))^2 = x^2/2 on the Scalar (ACT) engine.
                # The 1/2 folds the final *2 away: t = 2/(1+sum x^2)
                #                                    = 1/(0.5+sum x^2/2).
                xsq = tmp_pool.tile([P, blk, D], mybir.dt.float32, tag="xsq")
                nc.scalar.activation(
                    out=xsq, in_=x_t,
                    func=mybir.ActivationFunctionType.Square,
                    scale=0.7071067811865476,
                )

                # sq[p, b] = sum_d xsq[p, b, d] on Vector engine
                sq = small_pool.tile([P, blk], mybir.dt.float32, tag="sq")
                nc.vector.reduce_sum(out=sq, in_=xsq, axis=mybir.AxisListType.X)

                # u = sq + 0.5 as a 1x tensor_tensor (immune to the 2-port
                # perf-mode SBUF contention with the GpSimd multiply),
                # t = 1/u on DVE (ACT reciprocal is banned for accuracy).
                u = small_pool.tile([P, blk], mybir.dt.float32, tag="u")
                nc.vector.tensor_add(u, sq, half_b)
                t = small_pool.tile([P, blk], mybir.dt.float32, tag="t")
                nc.vector.reciprocal(out=t, in_=u)

                out_t = io_pool.tile([P, blk, E], mybir.dt.float32, tag="out")
                # out[:, :, D] = 1 - t on ACT: Copy(t * -1 + 1)
                nc.scalar.activation(
                    out=out_t[:, :, D], in_=t,
                    func=mybir.ActivationFunctionType.Copy,
                    bias=1.0, scale=-1.0,
                )
                # out[:, :, :D] = x * t (t broadcast along d)
                t_b = t[:, :].unsqueeze(2).broadcast_to([P, blk, D])
                if mul_engine == "gpsimd":
                    if i >= niter - 6 and blk >= 4:
                        # Pipeline tail: no DMA left to hide GpSimd's 3.6us
                        # multiply, so split blocks with the now-idle Vector
                        # engine to shorten the drain.
                        kv = blk // 4
                        nc.gpsimd.tensor_mul(
                            out_t[:, 0:blk - kv, 0:D],
                            x_t[:, 0:blk - kv, :],
                            t[:, 0:blk - kv].unsqueeze(2).broadcast_to(
                                [P, blk - kv, D]),
                        )
                        nc.vector.tensor_mul(
                            out_t[:, blk - kv:blk, 0:D],
                            x_t[:, blk - kv:blk, :],
                            t[:, blk - kv:blk].unsqueeze(2).broadcast_to(
                                [P, kv, D]),
                        )
                    else:
                        nc.gpsimd.tensor_mul(out_t[:, :, 0:D], x_t, t_b)
                else:
                    nc.vector.tensor_mul(out_t[:, :, 0:D], x_t, t_b)

                nc.sync.dma_start(out=ov[i], in_=out_t)

    nc.compile()
    return nc


_nc_cache: dict = {}


def _get_nc(rows: int = R, blk: int = 16):
    key = (rows, blk)
    if key not in _nc_cache:
        _nc_cache[key] = build_nc(rows, blk)
    return _nc_cache[key]


def kernel(x) -> np.ndarray:
    x = np.ascontiguousarray(np.asarray(x), dtype=np.float32)
    assert x.shape == (N, D), x.shape
    nc = _get_nc()
    shards = x.reshape(N_CORES, R, D)
    in_maps = [{"x": shards[c]} for c in range(N_CORES)]
    res = run_bass_kernel_spmd(nc, in_maps, core_ids=list(range(N_CORES)))
    return np.concatenate([r["out"] for r in res.results], axis=0)
